# revision 1
# baseline (speedup 1.0000x reference)
"""Self-contained Trainium2 kernel for nn_DynamicCrossAttention_40286793236903.

kernel(**inputs) takes the FULL inputs (as produced by setup_inputs) and
returns the FULL [4, 256, 64, 64] float32 output.

Sharding: pure data parallel over (batch, image-half): core ci handles
sample b=ci//2, output rows 32*(ci%2)..32*(ci%2)+31. One SPMD Bass program
runs on all 8 cores; all per-core variation is carried in the input data.

See build_nc for the on-device pipeline description.
"""
import numpy as np
from contextlib import ExitStack

import concourse.bass as bass
import concourse.mybir as mybir
import concourse.tile as tile
from concourse import bacc
from concourse.bass import AP, IndirectOffsetOnAxis
from concourse.bass_utils import run_bass_kernel_spmd
from concourse.masks import make_identity

F32 = mybir.dt.float32
BF16 = mybir.dt.bfloat16
I32 = mybir.dt.int32
I16 = mybir.dt.int16
ALU = mybir.AluOpType
ACTF = mybir.ActivationFunctionType

TAPS = [(ky, kx) for ky in (-1, 0, 1) for kx in (-1, 0, 1)]
C0 = -(16 * 64) - 16 + 1  # unshift +16 coords, +1 front pad row

_NC_CACHE = {}
LAST_RESULT = None


def build_nc(gdt=BF16):
    """One SPMD Bass program. gdt: dtype for conv/gather/einsum data."""
    nc = bacc.Bacc(None, target_bir_lowering=False, num_swdge_queues=4)

    tplp = nc.dram_tensor('tplp', [256, 23 * 32], F32, kind='ExternalInput')
    srch66 = nc.dram_tensor('srch66', [256, 34 * 66], F32, kind='ExternalInput')
    xrows = nc.dram_tensor('xrows', [4099, 256], F32, kind='ExternalInput')
    wpack = nc.dram_tensor('wpack', [128, 4 * 9 * 32], F32, kind='ExternalInput')
    dwpack = nc.dram_tensor('dwpack', [128, 9 * 2 * 2 * 128], F32, kind='ExternalInput')
    basei = nc.dram_tensor('basei', [128, 512], F32, kind='ExternalInput')
    mcomb = nc.dram_tensor('mcomb', [128, 4 * 3 * 18], F32, kind='ExternalInput')
    bias_comb = nc.dram_tensor('bias_comb', [18, 1], F32, kind='ExternalInput')
    bias_om = nc.dram_tensor('bias_om', [128, 1], F32, kind='ExternalInput')
    bias_out = nc.dram_tensor('bias_out', [256, 1], F32, kind='ExternalInput')
    rmaski = nc.dram_tensor('rmaski', [128, 2], F32, kind='ExternalInput')
    out = nc.dram_tensor('out', [256, 2048], F32, kind='ExternalOutput')

    with tile.TileContext(nc) as tc, ExitStack() as ctx:
        sb = ctx.enter_context(tc.tile_pool(name='sb', bufs=1))
        sbm = ctx.enter_context(tc.tile_pool(name='sbm', bufs=1))
        sbt = ctx.enter_context(tc.tile_pool(name='sbt', bufs=2))
        sba = ctx.enter_context(tc.tile_pool(name='sba', bufs=3))
        gpool = ctx.enter_context(tc.tile_pool(name='gpool', bufs=4))
        spool = ctx.enter_context(tc.tile_pool(name='spool', bufs=2))
        dram = ctx.enter_context(tc.tile_pool(name='dram', bufs=1, space='DRAM'))

        ident = sb.tile([128, 128], gdt, tag='ident', name='ident')
        make_identity(nc, ident[:])
        identf = sb.tile([128, 128], F32, tag='identf', name='identf')
        make_identity(nc, identf[:])

        wp = sb.tile([128, 4 * 9 * 32], gdt, tag='wp', name='wp')
        (nc.gpsimd if gdt != F32 else nc.sync).dma_start(wp[:], wpack[:])
        dw = sb.tile([128, 9 * 2 * 2 * 128], gdt, tag='dw', name='dw')
        (nc.gpsimd if gdt != F32 else nc.sync).dma_start(dw[:], dwpack[:])
        base_sb = sb.tile([128, 512], F32, tag='base', name='base')
        nc.sync.dma_start(base_sb[:], basei[:])
        mcomb_sb = sb.tile([128, 4 * 3 * 18], F32, tag='mcomb', name='mcomb')
        nc.sync.dma_start(mcomb_sb[:], mcomb[:])
        bcomb_sb = sb.tile([18, 1], F32, tag='bcomb', name='bcomb')
        nc.sync.dma_start(bcomb_sb[:], bias_comb[:])
        bom_sb = sb.tile([128, 1], F32, tag='bom', name='bom')
        nc.sync.dma_start(bom_sb[:], bias_om[:])
        bout_sb = sb.tile([128, 2], F32, tag='bout', name='bout')
        nc.sync.dma_start(bout_sb[:], bias_out[:].rearrange('(g p) o -> p (g o)', g=2))
        rmask_sb = sb.tile([128, 2], F32, tag='rmask', name='rmask')
        nc.sync.dma_start(rmask_sb[:], rmaski[:])

        if gdt == F32:
            xsrc = xrows
        else:
            x16 = dram.tile([4099, 256], gdt, name='x16')
            nc.gpsimd.dma_start(x16[:], xrows[:])
            xsrc = x16

        # ---- stage 1: upsample template + build combined [512ch, 34, 66] ----
        with tc.tile_pool(name='convsb', bufs=1) as convsb, \
             tc.tile_pool(name='psA', bufs=1, space='PSUM') as psA:
            comb = []
            for cg in range(2):
                tp = convsb.tile([128, 23 * 32], gdt, tag=f'tp{cg}', name=f'tp{cg}')
                (nc.gpsimd if gdt != F32 else nc.sync).dma_start(
                    tp[:], tplp[128 * cg:128 * (cg + 1), :])
                tp3 = tp[:].rearrange('p (r w) -> p r w', r=23)
                V = convsb.tile([128, 34 * 32], gdt, tag=f'vt{cg}', name=f'vt{cg}')
                V3 = V[:].rearrange('p (r w) -> p r w', r=34)
                tmp = convsb.tile([128, 34 * 32], gdt, tag=f'ut{cg}', name=f'ut{cg}')
                tmp3 = tmp[:].rearrange('p (r w) -> p r w', r=34)
                # vertical: V[i] = wa*tp[j] + wb*tp[j+1] (ts 4x + tt 2x; no stt)
                nc.vector.tensor_scalar_mul(tmp3[:, 0:16, :], tp3[:, 2:18, :], 0.25)
                nc.scalar.activation(V3[:, 2:34:2, :], tp3[:, 1:17, :], ACTF.Identity, scale=0.75)
                nc.vector.tensor_tensor(V3[:, 2:34:2, :], V3[:, 2:34:2, :], tmp3[:, 0:16, :], ALU.add)
                nc.vector.tensor_scalar_mul(tmp3[:, 0:16, :], tp3[:, 1:17, :], 0.75)
                nc.scalar.activation(V3[:, 1:33:2, :], tp3[:, 0:16, :], ACTF.Identity, scale=0.25)
                nc.vector.tensor_tensor(V3[:, 1:33:2, :], V3[:, 1:33:2, :], tmp3[:, 0:16, :], ALU.add)
                nc.vector.tensor_scalar_mul(tmp3[:, 0:1, :], tp3[:, 20:21, :], 0.25)
                nc.scalar.activation(V3[:, 0:1, :], tp3[:, 19:20, :], ACTF.Identity, scale=0.75)
                nc.vector.tensor_tensor(V3[:, 0:1, :], V3[:, 0:1, :], tmp3[:, 0:1, :], ALU.add)
                nc.vector.tensor_scalar_mul(tmp3[:, 0:1, :], tp3[:, 22:23, :], 0.75)
                nc.scalar.activation(V3[:, 33:34, :], tp3[:, 21:22, :], ACTF.Identity, scale=0.25)
                nc.vector.tensor_tensor(V3[:, 33:34, :], V3[:, 33:34, :], tmp3[:, 0:1, :], ALU.add)
                cb = convsb.tile([128, 34 * 66], gdt, tag=f'comb{cg}', name=f'comb{cg}')
                cb3 = cb[:].rearrange('p (r w) -> p r w', r=34)
                nc.vector.memset(cb[:], 0.0)
                h3 = tmp3
                nc.vector.tensor_scalar_mul(h3[:, :, 0:31], V3[:, :, 1:32], 0.75)
                nc.scalar.activation(cb3[:, :, 3:65:2], V3[:, :, 0:31], ACTF.Identity, scale=0.25)
                nc.vector.tensor_tensor(cb3[:, :, 3:65:2], cb3[:, :, 3:65:2], h3[:, :, 0:31], ALU.add)
                nc.vector.tensor_scalar_mul(h3[:, :, 0:31], V3[:, :, 1:32], 0.25)
                nc.scalar.activation(cb3[:, :, 2:64:2], V3[:, :, 0:31], ACTF.Identity, scale=0.75)
                nc.vector.tensor_tensor(cb3[:, :, 2:64:2], cb3[:, :, 2:64:2], h3[:, :, 0:31], ALU.add)
                nc.vector.tensor_copy(cb3[:, :, 1:2], V3[:, :, 0:1])
                nc.vector.tensor_copy(cb3[:, :, 64:65], V3[:, :, 31:32])
                comb.append(cb)
            for cg in range(2):
                cb = convsb.tile([128, 34 * 66], gdt, tag=f'comb{cg+2}', name=f'comb{cg+2}')
                (nc.gpsimd if gdt != F32 else nc.sync).dma_start(
                    cb[:], srch66[128 * cg:128 * (cg + 1), :])
                comb.append(cb)

            # ---- stage 2: offsets+mask conv, col-tiled quarters ----
            wp4 = wp[:].rearrange('p (g t m) -> p g t m', g=4, t=9)
            pom = psA.tile([128, 512], F32, name='pom')
            for q in range(4):
                first = True
                for gi, g in enumerate((2, 3, 0, 1)):
                    cb3 = comb[g][:].rearrange('p (r w) -> p r w', r=34)
                    for t, (ky, kx) in enumerate(TAPS):
                        rhs = cb3[:, 8 * q + 1 + ky: 8 * q + 9 + ky, 1 + kx: 65 + kx]
                        nc.tensor.matmul(
                            pom[32 * q:32 * q + 32, :], wp4[:, g, t, :], rhs,
                            start=first, stop=(gi == 3 and t == 8),
                            tile_position=(0, 32 * q))
                        first = False
            om = sb.tile([128, 512], F32, tag='om', name='om')
            nc.scalar.activation(om[:], pom[:], ACTF.Identity, bias=bom_sb[:], scale=1.0)

        sg = sb.tile([128, 512], F32, tag='sg', name='sg')
        nc.scalar.activation(sg[:], om[:], ACTF.Sigmoid)

        # ---- stage 3: index math (fp32, in-place tile reuse) ----
        def mtile(tag, dt=F32):
            return sbm.tile([128, 512], dt, tag=tag, name=tag)
        P = mtile('P')          # becomes Wf
        nc.vector.tensor_tensor(P[:], om[:], base_sb[:], ALU.add)
        nc.vector.tensor_scalar(P[:], P[:], 96.5, 14.0, ALU.min, ALU.max)
        T32 = mtile('T32', I32)
        nc.vector.tensor_copy(T32[:], P[:])
        Tf = mtile('Tf')        # becomes F (floor)
        nc.vector.tensor_copy(Tf[:], T32[:])
        Gg = mtile('Gg')        # becomes V0
        nc.vector.tensor_tensor(Gg[:], Tf[:], P[:], ALU.is_gt)
        nc.vector.tensor_tensor(Tf[:], Tf[:], Gg[:], ALU.subtract)
        nc.vector.tensor_tensor(P[:], P[:], Tf[:], ALU.subtract)
        Ff, Wf = Tf, P
        Vt = mtile('Vt')
        V0 = Gg
        nc.vector.tensor_scalar(V0[:], Ff[:], 16.0, None, ALU.is_ge)
        nc.vector.tensor_scalar(Vt[:], Ff[:], 80.0, None, ALU.is_lt)
        nc.vector.tensor_tensor(V0[:], V0[:], Vt[:], ALU.mult)
        V1 = mtile('V1')
        nc.vector.tensor_scalar(V1[:], Ff[:], 15.0, None, ALU.is_ge)
        nc.vector.tensor_scalar(Vt[:], Ff[:], 79.0, None, ALU.is_lt)
        nc.vector.tensor_tensor(V1[:], V1[:], Vt[:], ALU.mult)
        W0 = mtile('W0')
        nc.vector.tensor_scalar(W0[:], Wf[:], -1.0, 1.0, ALU.mult, ALU.add)
        nc.vector.tensor_tensor(W0[:], W0[:], V0[:], ALU.mult)
        W1 = Wf
        nc.vector.tensor_tensor(W1[:], Wf[:], V1[:], ALU.mult)
        AyT = V0
        nc.vector.tensor_scalar(AyT[:], Ff[:], 79.0, 16.0, ALU.min, ALU.max)
        AxC = V1
        nc.vector.tensor_scalar(AxC[:], Ff[:], 79.0, 15.0, ALU.min, ALU.max)
        AyB = Vt
        nc.vector.tensor_scalar(AyB[:], Ff[:], 78.0, 15.0, ALU.min, ALU.max)

        # ---- stage 4: idx assembly -> idxf [18, 2048] f32 (pixel-major) ----
        mc4 = mcomb_sb[:].rearrange('p (q j m) -> p q j m', q=4, j=3)
        idxf = sb.tile([18, 2048], F32, tag='idxf', name='idxf')
        with tc.tile_pool(name='psI', bufs=2, space='PSUM') as psI:
            for q in range(4):
                pidx = psI.tile([18, 512], F32, name='pidx')
                nc.tensor.matmul(pidx[:], mc4[:, q, 0, :], AyT[:], start=True, stop=False)
                nc.tensor.matmul(pidx[:], mc4[:, q, 1, :], AxC[:], start=False, stop=False)
                nc.tensor.matmul(pidx[:], mc4[:, q, 2, :], AyB[:], start=False, stop=True)
                nc.vector.tensor_scalar(idxf[:, 512 * q:512 * (q + 1)], pidx[:],
                                        bcomb_sb[:], None, ALU.add)
        # wrap for dma_gather: idx16[j, t*128 + 8*bb + a] = idxf[t, 128*bb + 16*a + j]
        # via PE transpose (px -> partitions) then identity-slice matmuls (base moves)
        idx16 = sb.tile([128, 18 * 128], I16, tag='idx16', name='idx16')
        tsb = sb.tile([128, 16 * 18], F32, tag='tsb', name='tsb')
        tsb3 = tsb[:].rearrange('p (b t) -> p b t', b=16)
        with tc.tile_pool(name='psT', bufs=2, space='PSUM') as psT:
            for bb in range(16):
                pT = psT.tile([128, 18], F32, name='pT')
                nc.tensor.transpose(pT[:], idxf[:, 128 * bb:128 * (bb + 1)],
                                    identf[0:18, 0:18], tile_position=(0, 0))
                nc.vector.tensor_copy(tsb3[:, bb, :], pT[:])
            for a in range(8):
                pW = psT.tile([16, 288], F32, name='pW')
                nc.tensor.matmul(pW[:], identf[:, 16 * a:16 * (a + 1)], tsb[:],
                                 start=True, stop=True)
                dsta = AP(idx16[:].tensor, idx16[:].offset + a,
                          [[18 * 128, 16], [8, 16], [128, 18]])
                nc.vector.tensor_copy(dsta, pW[:].rearrange('p (b t) -> p b t', b=16))
        nc.sync.dma_start(idx16[16:32, :], idx16[0:16, :])
        nc.sync.dma_start(idx16[32:64, :], idx16[0:32, :])
        nc.sync.dma_start(idx16[64:128, :], idx16[0:64, :])

        # ---- blend weights: mask-select, transpose, products ----
        for Wt in (W0, W1):
            nc.vector.tensor_scalar(Wt[:], Wt[:], rmask_sb[:, 0:1], None, ALU.mult)
            nc.vector.scalar_tensor_tensor(Wt[:], sg[:], rmask_sb[:, 1:2], Wt[:],
                                           ALU.mult, ALU.add)
        wprod = sb.tile([128, 16 * 6 * 9], F32, tag='wprod', name='wprod')
        wp3 = wprod[:].rearrange('p (b s t) -> p b s t', b=16, s=6)
        with tc.tile_pool(name='psW', bufs=2, space='PSUM') as psW:
            for b in range(16):
                q, cc = b // 4, b % 4
                pt = psW.tile([128, 54], F32, name='ptw')
                idq = identf[32 * q:32 * q + 27, 32 * q:32 * q + 27]
                nc.tensor.transpose(pt[:, 0:27], W0[32 * q:32 * q + 27, 128 * cc:128 * (cc + 1)],
                                    idq, tile_position=(32 * q, 0))
                nc.tensor.transpose(pt[:, 27:54], W1[32 * q:32 * q + 27, 128 * cc:128 * (cc + 1)],
                                    idq, tile_position=(32 * q, 0))
                ta = sbt.tile([128, 54], F32, tag='tall', name='tall')
                nc.vector.tensor_copy(ta[:], pt[:])
                r0 = sbt.tile([128, 9], F32, tag='r0', name='r0')
                nc.vector.tensor_tensor(r0[:], ta[:, 0:9], ta[:, 18:27], ALU.mult)
                r1 = sbt.tile([128, 9], F32, tag='r1', name='r1')
                nc.vector.tensor_tensor(r1[:], ta[:, 27:36], ta[:, 45:54], ALU.mult)
                nc.vector.tensor_tensor(wp3[:, b, 0, :], r0[:], ta[:, 9:18], ALU.mult)
                nc.vector.tensor_tensor(wp3[:, b, 1, :], r0[:], ta[:, 36:45], ALU.mult)
                nc.vector.tensor_tensor(wp3[:, b, 2, :], r1[:], ta[:, 9:18], ALU.mult)
                nc.vector.tensor_tensor(wp3[:, b, 3, :], r1[:], ta[:, 36:45], ALU.mult)

        # ---- stages 5-7: per-(tap, half) gather -> blend -> transpose ----
        ST = sb.tile([128, 18 * 2048], gdt, tag='ST', name='ST')
        ST3 = ST[:].rearrange('p (k n) -> p k n', k=18)
        xap = xsrc[:]
        with tc.tile_pool(name='psQ', bufs=2, space='PSUM') as psQ:
            for t in range(9):
                for hb in range(2):
                    gt = gpool.tile([128, 8, 512], gdt, tag='gt', name='gt')
                    gb = gpool.tile([128, 8, 512], gdt, tag='gt', name='gb')
                    inap = AP(xap.tensor, 0, [[256, 4098], [1, 512]])
                    for tb, gg in ((0, gt), (1, gb)):
                        icol = (9 * tb + t) * 128 + 64 * hb
                        qn = (t * 4 + hb * 2 + tb) % 4
                        nc.gpsimd.dma_gather(
                            out_ap=gg[:], in_ap=inap,
                            idxs_ap=idx16[:, icol:icol + 64],
                            num_idxs=1024, num_idxs_reg=1024,
                            elem_size=512, elem_step=256, queue_num=qn)
                    gt4 = gt[:]
                    gb4 = gb[:]
                    S = spool.tile([128, 8 * 256], gdt, tag='S', name='S')
                    S3 = S[:].rearrange('p (b n) -> p b n', b=8)
                    for blk in range(8):
                        b = 8 * hb + blk
                        m1 = sba.tile([128, 256], gdt, tag='m1', name='m1')
                        m2 = sba.tile([128, 256], gdt, tag='m2', name='m2')
                        m3 = sba.tile([128, 256], gdt, tag='m3', name='m3')
                        m4 = sba.tile([128, 256], gdt, tag='m4', name='m4')
                        nc.scalar.activation(m1[:], gt4[:, blk, 0:256], ACTF.Identity,
                                             scale=wp3[:, b, 0, t:t + 1])
                        nc.vector.tensor_scalar_mul(m2[:], gt4[:, blk, 256:512], wp3[:, b, 1, t:t + 1])
                        nc.scalar.activation(m3[:], gb4[:, blk, 0:256], ACTF.Identity,
                                             scale=wp3[:, b, 2, t:t + 1])
                        nc.vector.tensor_scalar_mul(m4[:], gb4[:, blk, 256:512], wp3[:, b, 3, t:t + 1])
                        nc.vector.tensor_tensor(m1[:], m1[:], m2[:], ALU.add)
                        nc.vector.tensor_tensor(m3[:], m3[:], m4[:], ALU.add)
                        nc.vector.tensor_tensor(S3[:, blk, :], m1[:], m3[:], ALU.add)
                    for half in range(2):
                        pq = psQ.tile([128, 1024], gdt, name='pq')
                        for j in range(4):
                            blk = 4 * half + j
                            for cg in range(2):
                                nc.tensor.transpose(
                                    pq[:, 256 * j + 128 * cg: 256 * j + 128 * (cg + 1)],
                                    S3[:, blk, 128 * cg:128 * (cg + 1)], ident[:])
                        pq4 = pq[:].rearrange('p (j g c) -> p j g c', j=4, g=2)
                        for cg in range(2):
                            dstp = ST3[:, 2 * t + cg,
                                       1024 * hb + 512 * half: 1024 * hb + 512 * (half + 1)]
                            nc.scalar.activation(
                                dstp.rearrange('p (j c) -> p j c', j=4), pq4[:, :, cg, :],
                                ACTF.Identity)

        # ---- stage 8: einsum + bias ----
        dw4 = dw[:].rearrange('p (k g o c) -> p k g o c', k=9, g=2, o=2)
        with tc.tile_pool(name='psO', bufs=2, space='PSUM') as psO:
            for og in range(2):
                for q in range(4):
                    po = psO.tile([128, 512], F32, name='po')
                    for t in range(9):
                        for cg in range(2):
                            nc.tensor.matmul(
                                po[:], dw4[:, t, cg, og, :],
                                ST3[:, 2 * t + cg, 512 * q:512 * (q + 1)],
                                start=(t == 0 and cg == 0), stop=(t == 8 and cg == 1))
                    osb = sbt.tile([128, 512], F32, tag='osb', name='osb')
                    nc.scalar.activation(osb[:], po[:], ACTF.Identity,
                                         bias=bout_sb[:, og:og + 1], scale=1.0)
                    nc.sync.dma_start(out[128 * og:128 * (og + 1), 512 * q:512 * (q + 1)], osb[:])

    nc.compile()
    return nc


def prep_core_inputs(inputs, b, h):
    tf = np.ascontiguousarray(np.asarray(inputs['template_feat'][b], dtype=np.float32))
    sf = np.ascontiguousarray(np.asarray(inputs['search_feat'][b], dtype=np.float32))
    offset_w = np.asarray(inputs['offset_w'], dtype=np.float32)
    offset_b = np.asarray(inputs['offset_b'], dtype=np.float32)
    mask_w = np.asarray(inputs['mask_w'], dtype=np.float32)
    mask_b = np.asarray(inputs['mask_b'], dtype=np.float32)
    deform_w = np.asarray(inputs['deform_w'], dtype=np.float32)
    deform_b = np.asarray(inputs['deform_b'], dtype=np.float32)

    tplp = np.zeros((256, 23, 32), np.float32)
    for j in range(19):
        tplp[:, j] = tf[:, min(max(16 * h - 1 + j, 0), 31)]
    if h == 0:
        tplp[:, 21] = tf[:, 15]
        tplp[:, 22] = tf[:, 16]
    else:
        tplp[:, 19] = tf[:, 15]
        tplp[:, 20] = tf[:, 16]

    srch66 = np.zeros((256, 34, 66), np.float32)
    for i in range(34):
        r = 32 * h - 1 + i
        if 0 <= r <= 63:
            srch66[:, i, 1:65] = sf[:, r]

    xrows = np.zeros((4099, 256), np.float32)
    xrows[1:4097] = sf.reshape(256, 4096).T

    wpack = np.zeros((128, 4, 9, 32), np.float32)
    for g in range(4):
        for t, (ky, kx) in enumerate(TAPS):
            cs = slice(128 * g, 128 * (g + 1))
            wpack[:, g, t, 0:9] = offset_w[0::2, cs, ky + 1, kx + 1].T
            wpack[:, g, t, 9:18] = offset_w[1::2, cs, ky + 1, kx + 1].T
            if ky == 0 and kx == 0:
                wpack[:, g, t, 18:27] = mask_w[:, cs, 0, 0].T
    wk = deform_w.reshape(256, 256, 3, 3)
    dwp = np.zeros((128, 9, 2, 2, 128), np.float32)
    for t in range(9):
        ky, kx = TAPS[t]
        for cg in range(2):
            for og in range(2):
                dwp[:, t, cg, og, :] = wk[128 * og:128 * (og + 1),
                                          128 * cg:128 * (cg + 1), ky + 1, kx + 1].T

    basei = np.zeros((128, 512), np.float32)
    col = np.arange(512)
    for q in range(4):
        for m in range(9):
            basei[32 * q + m] = 32 * h + 8 * q + col // 64 + TAPS[m][0] + 16
            basei[32 * q + 9 + m] = col % 64 + TAPS[m][1] + 16

    mcomb = np.zeros((128, 4, 3, 18), np.float32)
    for q in range(4):
        for t in range(9):
            mcomb[32 * q + t, q, 0, t] = 64.0
            mcomb[32 * q + 9 + t, q, 1, t] = 1.0
            mcomb[32 * q + 9 + t, q, 1, 9 + t] = 1.0
            mcomb[32 * q + t, q, 2, 9 + t] = 64.0
    bias_comb = np.full((18, 1), float(C0), np.float32)
    bias_comb[9:] += 64.0

    bias_om = np.zeros((128, 1), np.float32)
    for q in range(4):
        bias_om[32 * q + 0:32 * q + 9, 0] = offset_b[0::2]
        bias_om[32 * q + 9:32 * q + 18, 0] = offset_b[1::2]
        bias_om[32 * q + 18:32 * q + 27, 0] = mask_b

    rmaski = np.zeros((128, 2), np.float32)
    for q in range(4):
        rmaski[32 * q:32 * q + 18, 0] = 1.0
        rmaski[32 * q + 18:32 * q + 32, 1] = 1.0

    return {
        'rmaski': rmaski,
        'tplp': tplp.reshape(256, 23 * 32),
        'srch66': srch66.reshape(256, 34 * 66),
        'xrows': xrows,
        'wpack': wpack.reshape(128, 4 * 9 * 32),
        'dwpack': dwp.reshape(128, 9 * 2 * 2 * 128),
        'basei': basei,
        'mcomb': mcomb.reshape(128, 4 * 3 * 18),
        'bias_comb': bias_comb,
        'bias_om': bias_om,
        'bias_out': deform_b.reshape(256, 1).astype(np.float32),
    }


def kernel(**inputs):
    key = 'bf16'
    if key not in _NC_CACHE:
        _NC_CACHE[key] = build_nc(gdt=BF16)
    nc = _NC_CACHE[key]
    in_maps = [prep_core_inputs(inputs, ci // 2, ci % 2) for ci in range(8)]
    res = run_bass_kernel_spmd(nc, in_maps, core_ids=list(range(8)))
    global LAST_RESULT
    LAST_RESULT = res
    out = np.zeros((4, 256, 64, 64), np.float32)
    for ci in range(8):
        b, h = ci // 2, ci % 2
        out[b][:, 32 * h:32 * h + 32, :] = res.results[ci]['out'].reshape(256, 32, 64)
    return out



# revision 10
# speedup vs baseline: 1.1031x; 1.1031x over previous
"""Self-contained Trainium2 kernel for nn_DynamicCrossAttention_40286793236903.

kernel(**inputs) takes the FULL inputs (as produced by setup_inputs) and
returns the FULL [4, 256, 64, 64] float32 output.

Sharding: pure data parallel over (batch, image-half): core ci handles
sample b=ci//2, output rows 32*(ci%2)..32*(ci%2)+31. One SPMD Bass program
runs on all 8 cores; all per-core variation is carried in the input data.

Pipeline per core (2048 output pixels):
  1. bilinear-upsample template half (+halo) -> combined [512ch, 34, 66]
  2. offsets+mask 3x3 conv via 36 accumulating matmuls -> om [128, 512]
     (per quarter q: rows 32q+0..8 = dy taps, +9..17 = dx, +18..26 = mask)
  3. index math: py/px = coord+offset clamped to [-1, 64.996]; floor/frac
  4. idx = 66*(y0+1) + (x0+1) into the quad table; wrap to idx16 for
     dma_gather; corner weights wq = bilinear products * sigmoid(mask)
  5. per tap t, half hb: dma_gather 1024 quad rows (2KB each: all 4
     corners of a position, zero-padded at borders -> no validity masks)
  6. blend: per 128-pixel block: ACT start (corner a * wa) then 3 DVE
     scalar_tensor_tensor fused mult-adds -> S [128px, 256ch] bf16
  7. PE-transpose S to channel-major ST [128ch, 18, 2048px]
  8. einsum out[o,p] = sum_{k,cg} dw[o,:,k] @ ST[:,k(cg),p]; og=0
     accumulated inline per tap, og=1 as a tail pass
"""
import numpy as np
from contextlib import ExitStack

import concourse.bass as bass
import concourse.mybir as mybir
import concourse.tile as tile
from concourse import bacc
from concourse.bass import AP
from concourse.bass_utils import run_bass_kernel_spmd
from concourse.masks import make_identity

F32 = mybir.dt.float32
BF16 = mybir.dt.bfloat16
I32 = mybir.dt.int32
I16 = mybir.dt.int16
ALU = mybir.AluOpType
ACTF = mybir.ActivationFunctionType

TAPS = [(ky, kx) for ky in (-1, 0, 1) for kx in (-1, 0, 1)]
CLAMP_LO = -1.0
CLAMP_HI = 64.99609375

_NC_CACHE = {}
LAST_RESULT = None


def build_nc():
    nc = bacc.Bacc(None, target_bir_lowering=False, num_swdge_queues=4)

    tplp = nc.dram_tensor('tplp', [256, 23 * 32], BF16, kind='ExternalInput')
    srch66 = nc.dram_tensor('srch66', [256, 34 * 66], BF16, kind='ExternalInput')
    qtab = nc.dram_tensor('qtab', [4356, 1024], BF16, kind='ExternalInput')
    wpack = nc.dram_tensor('wpack', [128, 4 * 9 * 32], BF16, kind='ExternalInput')
    dwpack = nc.dram_tensor('dwpack', [128, 9 * 2 * 2 * 128], BF16, kind='ExternalInput')
    basei = nc.dram_tensor('basei', [128, 512], F32, kind='ExternalInput')
    mcomb = nc.dram_tensor('mcomb', [128, 4 * 9], F32, kind='ExternalInput')
    bias_om = nc.dram_tensor('bias_om', [128, 1], F32, kind='ExternalInput')
    bias_out = nc.dram_tensor('bias_out', [256, 1], F32, kind='ExternalInput')
    out = nc.dram_tensor('out', [256, 2048], F32, kind='ExternalOutput')

    with tile.TileContext(nc) as tc, ExitStack() as ctx:
        sb = ctx.enter_context(tc.tile_pool(name='sb', bufs=1))
        sbm = ctx.enter_context(tc.tile_pool(name='sbm', bufs=1))
        sbt = ctx.enter_context(tc.tile_pool(name='sbt', bufs=2))
        sba = ctx.enter_context(tc.tile_pool(name='sba', bufs=4))
        gpool = ctx.enter_context(tc.tile_pool(name='gpool', bufs=3))
        spool = ctx.enter_context(tc.tile_pool(name='spool', bufs=2))

        ident = sb.tile([128, 128], BF16, tag='ident', name='ident')
        make_identity(nc, ident[:])
        identf = sb.tile([128, 128], F32, tag='identf', name='identf')
        make_identity(nc, identf[:])

        wp = sb.tile([128, 4 * 9 * 32], BF16, tag='wp', name='wp')
        nc.sync.dma_start(wp[:], wpack[:])
        dw = sb.tile([128, 9 * 2 * 2 * 128], BF16, tag='dw', name='dw')
        nc.sync.dma_start(dw[:], dwpack[:])
        base_sb = sb.tile([128, 512], F32, tag='base', name='base')
        nc.sync.dma_start(base_sb[:], basei[:])
        mc_sb = sb.tile([128, 4 * 9], F32, tag='mcomb', name='mcomb')
        nc.sync.dma_start(mc_sb[:], mcomb[:])
        bom_sb = sb.tile([128, 1], F32, tag='bom', name='bom')
        nc.sync.dma_start(bom_sb[:], bias_om[:])
        bout_sb = sb.tile([128, 2], F32, tag='bout', name='bout')
        nc.sync.dma_start(bout_sb[:], bias_out[:].rearrange('(g p) o -> p (g o)', g=2))

        # ---- stage 1: upsample template + build combined [512ch, 34, 66] ----
        with tc.tile_pool(name='convsb', bufs=1) as convsb, \
             tc.tile_pool(name='psA', bufs=1, space='PSUM') as psA:
            comb = []
            for cg in range(2):
                tp = convsb.tile([128, 23 * 32], BF16, tag=f'tp{cg}', name=f'tp{cg}')
                nc.sync.dma_start(tp[:], tplp[128 * cg:128 * (cg + 1), :])
                tp3 = tp[:].rearrange('p (r w) -> p r w', r=23)
                V = convsb.tile([128, 34 * 32], BF16, tag=f'vt{cg}', name=f'vt{cg}')
                V3 = V[:].rearrange('p (r w) -> p r w', r=34)
                tmp = convsb.tile([128, 34 * 32], BF16, tag=f'ut{cg}', name=f'ut{cg}')
                tmp3 = tmp[:].rearrange('p (r w) -> p r w', r=34)
                nc.vector.tensor_scalar_mul(tmp3[:, 0:16, :], tp3[:, 2:18, :], 0.25)
                nc.scalar.activation(V3[:, 2:34:2, :], tp3[:, 1:17, :], ACTF.Identity, scale=0.75)
                nc.vector.tensor_tensor(V3[:, 2:34:2, :], V3[:, 2:34:2, :], tmp3[:, 0:16, :], ALU.add)
                nc.vector.tensor_scalar_mul(tmp3[:, 0:16, :], tp3[:, 1:17, :], 0.75)
                nc.scalar.activation(V3[:, 1:33:2, :], tp3[:, 0:16, :], ACTF.Identity, scale=0.25)
                nc.vector.tensor_tensor(V3[:, 1:33:2, :], V3[:, 1:33:2, :], tmp3[:, 0:16, :], ALU.add)
                nc.vector.tensor_scalar_mul(tmp3[:, 0:1, :], tp3[:, 20:21, :], 0.25)
                nc.scalar.activation(V3[:, 0:1, :], tp3[:, 19:20, :], ACTF.Identity, scale=0.75)
                nc.vector.tensor_tensor(V3[:, 0:1, :], V3[:, 0:1, :], tmp3[:, 0:1, :], ALU.add)
                nc.vector.tensor_scalar_mul(tmp3[:, 0:1, :], tp3[:, 22:23, :], 0.75)
                nc.scalar.activation(V3[:, 33:34, :], tp3[:, 21:22, :], ACTF.Identity, scale=0.25)
                nc.vector.tensor_tensor(V3[:, 33:34, :], V3[:, 33:34, :], tmp3[:, 0:1, :], ALU.add)
                cb = convsb.tile([128, 34 * 66], BF16, tag=f'comb{cg}', name=f'comb{cg}')
                cb3 = cb[:].rearrange('p (r w) -> p r w', r=34)
                nc.vector.memset(cb[:], 0.0)
                h3 = tmp3
                nc.vector.tensor_scalar_mul(h3[:, :, 0:31], V3[:, :, 1:32], 0.75)
                nc.scalar.activation(cb3[:, :, 3:65:2], V3[:, :, 0:31], ACTF.Identity, scale=0.25)
                nc.vector.tensor_tensor(cb3[:, :, 3:65:2], cb3[:, :, 3:65:2], h3[:, :, 0:31], ALU.add)
                nc.vector.tensor_scalar_mul(h3[:, :, 0:31], V3[:, :, 1:32], 0.25)
                nc.scalar.activation(cb3[:, :, 2:64:2], V3[:, :, 0:31], ACTF.Identity, scale=0.75)
                nc.vector.tensor_tensor(cb3[:, :, 2:64:2], cb3[:, :, 2:64:2], h3[:, :, 0:31], ALU.add)
                nc.vector.tensor_copy(cb3[:, :, 1:2], V3[:, :, 0:1])
                nc.vector.tensor_copy(cb3[:, :, 64:65], V3[:, :, 31:32])
                comb.append(cb)
            for cg in range(2):
                cb = convsb.tile([128, 34 * 66], BF16, tag=f'comb{cg+2}', name=f'comb{cg+2}')
                nc.sync.dma_start(cb[:], srch66[128 * cg:128 * (cg + 1), :])
                comb.append(cb)

            # ---- stage 2: offsets+mask conv, col-tiled quarters ----
            wp4 = wp[:].rearrange('p (g t m) -> p g t m', g=4, t=9)
            pom = psA.tile([128, 512], F32, name='pom')
            for q in range(4):
                first = True
                for gi, g in enumerate((2, 3, 0, 1)):
                    cb3 = comb[g][:].rearrange('p (r w) -> p r w', r=34)
                    for t, (ky, kx) in enumerate(TAPS):
                        rhs = cb3[:, 8 * q + 1 + ky: 8 * q + 9 + ky, 1 + kx: 65 + kx]
                        nc.tensor.matmul(
                            pom[32 * q:32 * q + 32, :], wp4[:, g, t, :], rhs,
                            start=first, stop=(gi == 3 and t == 8),
                            tile_position=(0, 32 * q))
                        first = False
            om = sb.tile([128, 512], F32, tag='om', name='om')
            nc.scalar.activation(om[:], pom[:], ACTF.Identity, bias=bom_sb[:], scale=1.0)

        sg = sb.tile([128, 512], F32, tag='sg', name='sg')
        nc.scalar.activation(sg[:], om[:], ACTF.Sigmoid)

        # ---- stage 3: index math (fp32) ----
        def mtile(tag, dt=F32):
            return sbm.tile([128, 512], dt, tag=tag, name=tag)
        P = mtile('P')
        nc.vector.tensor_tensor(P[:], om[:], base_sb[:], ALU.add)
        nc.vector.tensor_scalar(P[:], P[:], CLAMP_HI, CLAMP_LO, ALU.min, ALU.max)
        T32 = mtile('T32', I32)
        nc.vector.tensor_copy(T32[:], P[:])
        Tf = mtile('Tf')
        nc.vector.tensor_copy(Tf[:], T32[:])
        Gg = mtile('Gg')
        nc.vector.tensor_tensor(Gg[:], Tf[:], P[:], ALU.is_gt)
        nc.vector.tensor_tensor(Tf[:], Tf[:], Gg[:], ALU.subtract)   # floor
        nc.vector.tensor_tensor(P[:], P[:], Tf[:], ALU.subtract)     # frac
        Wf = P

        # ---- stage 4a: idxf [9, 2048] = 66*(y0+1) + (x0+1) ----
        idxf = sb.tile([9, 2048], F32, tag='idxf', name='idxf')
        with tc.tile_pool(name='psI', bufs=2, space='PSUM') as psI:
            for q in range(4):
                pidx = psI.tile([9, 512], F32, name='pidx')
                nc.tensor.matmul(pidx[:], mc_sb[:, 9 * q:9 * (q + 1)], Tf[:],
                                 start=True, stop=True)
                nc.vector.tensor_scalar(idxf[:, 512 * q:512 * (q + 1)], pidx[:],
                                        67.0, None, ALU.add)
        # wrap: idx16[j, 128*t + 8*bb + a] = idxf[t, 128*bb + 16*a + j]
        idx16 = sb.tile([128, 9 * 128], I16, tag='idx16', name='idx16')
        tsb = sb.tile([128, 16 * 9], F32, tag='tsb', name='tsb')
        tsb3 = tsb[:].rearrange('p (b t) -> p b t', b=16)
        with tc.tile_pool(name='psT', bufs=2, space='PSUM') as psT:
            for bb in range(16):
                pT = psT.tile([128, 9], F32, name='pT')
                nc.tensor.transpose(pT[:], idxf[:, 128 * bb:128 * (bb + 1)],
                                    identf[0:9, 0:9], tile_position=(0, 0))
                nc.vector.tensor_copy(tsb3[:, bb, :], pT[:])
            for a in range(8):
                pW = psT.tile([16, 144], F32, name='pW')
                nc.tensor.matmul(pW[:], identf[:, 16 * a:16 * (a + 1)], tsb[:],
                                 start=True, stop=True)
                dsta = AP(idx16[:].tensor, idx16[:].offset + a,
                          [[9 * 128, 16], [8, 16], [128, 9]])
                nc.vector.tensor_copy(dsta, pW[:].rearrange('p (b t) -> p b t', b=16))
        nc.sync.dma_start(idx16[16:32, :], idx16[0:16, :])
        nc.sync.dma_start(idx16[32:64, :], idx16[0:32, :])
        nc.sync.dma_start(idx16[64:128, :], idx16[0:64, :])

        # ---- stage 4b: corner weights wq[128px, bb, j, t] ----
        wq = sb.tile([128, 16 * 4 * 9], F32, tag='wq', name='wq')
        wq4 = wq[:].rearrange('p (b j t) -> p b j t', b=16, j=4)
        with tc.tile_pool(name='psW', bufs=2, space='PSUM') as psW:
            for bb in range(16):
                q, cc = bb // 4, bb % 4
                pt = psW.tile([128, 45], F32, name='ptw')
                idq18 = identf[32 * q:32 * q + 18, 32 * q:32 * q + 18]
                idq27 = identf[32 * q:32 * q + 27, 32 * q:32 * q + 27]
                nc.tensor.transpose(pt[:, 0:18],
                                    Wf[32 * q:32 * q + 18, 128 * cc:128 * (cc + 1)],
                                    idq18, tile_position=(32 * q, 0))
                nc.tensor.transpose(pt[:, 18:45],
                                    sg[32 * q:32 * q + 27, 128 * cc:128 * (cc + 1)],
                                    idq27, tile_position=(32 * q, 0))
                ta = sbt.tile([128, 45], F32, tag='tall', name='tall')
                nc.vector.tensor_copy(ta[:], pt[:])
                # ta cols: 0:9 = wy, 9:18 = wx, 36:45 = sigmoid(mask)
                iwy = sbt.tile([128, 9], F32, tag='iwy', name='iwy')
                nc.vector.tensor_scalar(iwy[:], ta[:, 0:9], -1.0, 1.0, ALU.mult, ALU.add)
                iwx = sbt.tile([128, 9], F32, tag='iwx', name='iwx')
                nc.vector.tensor_scalar(iwx[:], ta[:, 9:18], -1.0, 1.0, ALU.mult, ALU.add)
                q1 = sbt.tile([128, 9], F32, tag='q1', name='q1')
                nc.vector.tensor_tensor(q1[:], iwy[:], ta[:, 36:45], ALU.mult)
                q2 = sbt.tile([128, 9], F32, tag='q2', name='q2')
                nc.vector.tensor_tensor(q2[:], ta[:, 0:9], ta[:, 36:45], ALU.mult)
                nc.vector.tensor_tensor(wq4[:, bb, 0, :], q1[:], iwx[:], ALU.mult)
                nc.vector.tensor_tensor(wq4[:, bb, 1, :], q1[:], ta[:, 9:18], ALU.mult)
                nc.vector.tensor_tensor(wq4[:, bb, 2, :], q2[:], iwx[:], ALU.mult)
                nc.vector.tensor_tensor(wq4[:, bb, 3, :], q2[:], ta[:, 9:18], ALU.mult)

        # ---- stages 5-8: per image half: gather -> blend -> transpose ->
        #      einsum (og0 inline per tap, og1 per half as tail) ----
        ST = sb.tile([128, 18 * 1024], BF16, tag='ST', name='ST')
        ST3 = ST[:].rearrange('p (k n) -> p k n', k=18)
        inap = AP(qtab[:].tensor, 0, [[1024, 4356], [1, 1024]])
        dw4 = dw[:].rearrange('p (k g o c) -> p k g o c', k=9, g=2, o=2)
        with tc.tile_pool(name='psQ', bufs=2, space='PSUM') as psQ, \
             tc.tile_pool(name='psO', bufs=1, space='PSUM') as psO, \
             tc.tile_pool(name='psO1', bufs=2, space='PSUM') as psO1:
            for hb in range(2):
                po0 = [psO.tile([128, 512], F32, tag=f'po0_{i}', name=f'po0_{i}')
                       for i in range(2)]
                for t in range(9):
                    g = gpool.tile([128, 8, 1024], BF16, tag='gt', name='gt')
                    nc.gpsimd.dma_gather(
                        out_ap=g[:], in_ap=inap,
                        idxs_ap=idx16[:, 128 * t + 64 * hb:128 * t + 64 * (hb + 1)],
                        num_idxs=1024, num_idxs_reg=1024,
                        elem_size=1024, elem_step=1024,
                        queue_num=(9 * hb + t) % 4)
                    g3 = g[:]
                    S = spool.tile([128, 8, 256], BF16, tag='S', name='S')
                    S3 = S[:]
                    for blk in range(8):
                        bb = 8 * hb + blk
                        m = sba.tile([128, 256], BF16, tag='m', name='m')
                        nc.scalar.activation(m[:], g3[:, blk, 0:256], ACTF.Identity,
                                             scale=wq4[:, bb, 0, t:t + 1])
                        nc.vector.scalar_tensor_tensor(
                            m[:], g3[:, blk, 256:512], wq4[:, bb, 1, t:t + 1], m[:],
                            ALU.mult, ALU.add)
                        nc.vector.scalar_tensor_tensor(
                            m[:], g3[:, blk, 512:768], wq4[:, bb, 2, t:t + 1], m[:],
                            ALU.mult, ALU.add)
                        nc.vector.scalar_tensor_tensor(
                            S3[:, blk, :], g3[:, blk, 768:1024], wq4[:, bb, 3, t:t + 1],
                            m[:], ALU.mult, ALU.add)
                    for cg in range(2):
                        pq = psQ.tile([128, 1024], BF16, name='pq')
                        for blk in range(8):
                            nc.tensor.transpose(
                                pq[:, 128 * blk:128 * (blk + 1)],
                                S3[:, blk, 128 * cg:128 * (cg + 1)], ident[:])
                        nc.scalar.activation(ST3[:, 2 * t + cg, :], pq[:],
                                             ACTF.Identity)
                        for i in range(2):
                            nc.tensor.matmul(
                                po0[i][:], dw4[:, t, cg, 0, :],
                                ST3[:, 2 * t + cg, 512 * i:512 * (i + 1)],
                                start=(t == 0 and cg == 0), stop=(t == 8 and cg == 1))
                for i in range(2):
                    q = 2 * hb + i
                    osb = sbt.tile([128, 512], F32, tag='osb', name='osb')
                    nc.scalar.activation(osb[:], po0[i][:], ACTF.Identity,
                                         bias=bout_sb[:, 0:1], scale=1.0)
                    nc.sync.dma_start(out[0:128, 512 * q:512 * (q + 1)], osb[:])
                    po = psO1.tile([128, 512], F32, name='po')
                    for t in range(9):
                        for cg in range(2):
                            nc.tensor.matmul(
                                po[:], dw4[:, t, cg, 1, :],
                                ST3[:, 2 * t + cg, 512 * i:512 * (i + 1)],
                                start=(t == 0 and cg == 0), stop=(t == 8 and cg == 1))
                    osb1 = sbt.tile([128, 512], F32, tag='osb1', name='osb1')
                    nc.scalar.activation(osb1[:], po[:], ACTF.Identity,
                                         bias=bout_sb[:, 1:2], scale=1.0)
                    nc.sync.dma_start(out[128:256, 512 * q:512 * (q + 1)], osb1[:])

    nc.compile()
    return nc


def _bf16(x):
    import ml_dtypes
    return np.asarray(x, dtype=np.float32).astype(ml_dtypes.bfloat16)


def prep_sample(inputs, b):
    """Per-sample (shared by both h-halves) heavy prep: the quad table."""
    sf = np.ascontiguousarray(np.asarray(inputs['search_feat'][b], dtype=np.float32))
    P = np.zeros((67, 67, 256), np.float32)
    P[1:65, 1:65] = sf.transpose(1, 2, 0)
    Q = np.concatenate([P[:66, :66], P[:66, 1:67], P[1:67, :66], P[1:67, 1:67]],
                       axis=-1)
    return _bf16(Q.reshape(4356, 1024))


def prep_core_inputs(inputs, b, h, qtab):
    tf = np.ascontiguousarray(np.asarray(inputs['template_feat'][b], dtype=np.float32))
    sf = np.ascontiguousarray(np.asarray(inputs['search_feat'][b], dtype=np.float32))
    offset_w = np.asarray(inputs['offset_w'], dtype=np.float32)
    offset_b = np.asarray(inputs['offset_b'], dtype=np.float32)
    mask_w = np.asarray(inputs['mask_w'], dtype=np.float32)
    mask_b = np.asarray(inputs['mask_b'], dtype=np.float32)
    deform_w = np.asarray(inputs['deform_w'], dtype=np.float32)
    deform_b = np.asarray(inputs['deform_b'], dtype=np.float32)

    tplp = np.zeros((256, 23, 32), np.float32)
    for j in range(19):
        tplp[:, j] = tf[:, min(max(16 * h - 1 + j, 0), 31)]
    if h == 0:
        tplp[:, 21] = tf[:, 15]
        tplp[:, 22] = tf[:, 16]
    else:
        tplp[:, 19] = tf[:, 15]
        tplp[:, 20] = tf[:, 16]

    srch66 = np.zeros((256, 34, 66), np.float32)
    for i in range(34):
        r = 32 * h - 1 + i
        if 0 <= r <= 63:
            srch66[:, i, 1:65] = sf[:, r]

    wpack = np.zeros((128, 4, 9, 32), np.float32)
    for g in range(4):
        for t, (ky, kx) in enumerate(TAPS):
            cs = slice(128 * g, 128 * (g + 1))
            wpack[:, g, t, 0:9] = offset_w[0::2, cs, ky + 1, kx + 1].T
            wpack[:, g, t, 9:18] = offset_w[1::2, cs, ky + 1, kx + 1].T
            if ky == 0 and kx == 0:
                wpack[:, g, t, 18:27] = mask_w[:, cs, 0, 0].T
    wk = deform_w.reshape(256, 256, 3, 3)
    dwp = np.zeros((128, 9, 2, 2, 128), np.float32)
    for t in range(9):
        ky, kx = TAPS[t]
        for cg in range(2):
            for og in range(2):
                dwp[:, t, cg, og, :] = wk[128 * og:128 * (og + 1),
                                          128 * cg:128 * (cg + 1), ky + 1, kx + 1].T

    basei = np.zeros((128, 512), np.float32)
    col = np.arange(512)
    for q in range(4):
        for m in range(9):
            basei[32 * q + m] = 32 * h + 8 * q + col // 64 + TAPS[m][0]
            basei[32 * q + 9 + m] = col % 64 + TAPS[m][1]

    mcomb = np.zeros((128, 4, 9), np.float32)
    for q in range(4):
        for t in range(9):
            mcomb[32 * q + t, q, t] = 66.0
            mcomb[32 * q + 9 + t, q, t] = 1.0

    bias_om = np.zeros((128, 1), np.float32)
    for q in range(4):
        bias_om[32 * q + 0:32 * q + 9, 0] = offset_b[0::2]
        bias_om[32 * q + 9:32 * q + 18, 0] = offset_b[1::2]
        bias_om[32 * q + 18:32 * q + 27, 0] = mask_b

    return {
        'tplp': _bf16(tplp.reshape(256, 23 * 32)),
        'srch66': _bf16(srch66.reshape(256, 34 * 66)),
        'qtab': qtab,
        'wpack': _bf16(wpack.reshape(128, 4 * 9 * 32)),
        'dwpack': _bf16(dwp.reshape(128, 9 * 2 * 2 * 128)),
        'basei': basei,
        'mcomb': mcomb.reshape(128, 4 * 9),
        'bias_om': bias_om,
        'bias_out': deform_b.reshape(256, 1).astype(np.float32),
    }


def kernel(**inputs):
    key = 'v2'
    if key not in _NC_CACHE:
        _NC_CACHE[key] = build_nc()
    nc = _NC_CACHE[key]
    qtabs = [prep_sample(inputs, b) for b in range(4)]
    in_maps = [prep_core_inputs(inputs, ci // 2, ci % 2, qtabs[ci // 2])
               for ci in range(8)]
    res = run_bass_kernel_spmd(nc, in_maps, core_ids=list(range(8)))
    global LAST_RESULT
    LAST_RESULT = res
    out = np.zeros((4, 256, 64, 64), np.float32)
    for ci in range(8):
        b, h = ci // 2, ci % 2
        out[b][:, 32 * h:32 * h + 32, :] = res.results[ci]['out'].reshape(256, 32, 64)
    return out


# revision 13
# speedup vs baseline: 1.2921x; 1.1713x over previous
"""Self-contained Trainium2 kernel for nn_DynamicCrossAttention_40286793236903.

kernel(**inputs) takes the FULL inputs (as produced by setup_inputs) and
returns the FULL [4, 256, 64, 64] float32 output.

Sharding: pure data parallel over (batch, image-half): core ci handles
sample b=ci//2, output rows 32*(ci%2)..32*(ci%2)+31. One SPMD Bass program
runs on all 8 cores; all per-core variation is carried in the input data.

Pipeline per core (2048 output pixels):
  1. bilinear-upsample template half (+halo) -> combined [512ch, 34, 66]
  2. offsets+mask 3x3 conv via 36 accumulating matmuls -> om [128, 512]
     (per quarter q: rows 32q+0..8 = dy taps, +9..17 = dx, +18..26 = mask)
  3. index math: py/px = coord+offset clamped to [-1, 64.996]; floor/frac
  4. idx = 66*(y0+1) + (x0+1) into the quad table; wrap to idx16 for
     dma_gather; corner weights wq = bilinear products * sigmoid(mask)
  5. per tap t, half hb: dma_gather 1024 quad rows (2KB each: all 4
     corners of a position, zero-padded at borders -> no validity masks)
  6. blend: per 128-pixel block: ACT start (corner a * wa) then 3 DVE
     scalar_tensor_tensor fused mult-adds -> S [128px, 256ch] bf16
  7. PE-transpose S to channel-major ST [128ch, 18, 2048px]
  8. einsum out[o,p] = sum_{k,cg} dw[o,:,k] @ ST[:,k(cg),p]; og=0
     accumulated inline per tap, og=1 as a tail pass
"""
import numpy as np
from contextlib import ExitStack

import concourse.bass as bass
import concourse.mybir as mybir
import concourse.tile as tile
from concourse import bacc
from concourse.bass import AP
from concourse.bass_utils import run_bass_kernel_spmd
from concourse.masks import make_identity

F32 = mybir.dt.float32
BF16 = mybir.dt.bfloat16
I32 = mybir.dt.int32
I16 = mybir.dt.int16
ALU = mybir.AluOpType
ACTF = mybir.ActivationFunctionType

TAPS = [(ky, kx) for ky in (-1, 0, 1) for kx in (-1, 0, 1)]
CLAMP_LO = -1.0
CLAMP_HI = 64.99609375

_NC_CACHE = {}
LAST_RESULT = None


def build_nc():
    nc = bacc.Bacc(None, target_bir_lowering=False, num_swdge_queues=4)

    tplp = nc.dram_tensor('tplp', [256, 23 * 32], BF16, kind='ExternalInput')
    srch66 = nc.dram_tensor('srch66', [256, 34 * 66], BF16, kind='ExternalInput')
    qtab = nc.dram_tensor('qtab', [4356, 1024], BF16, kind='ExternalInput')
    wpack = nc.dram_tensor('wpack', [128, 4 * 9 * 32], BF16, kind='ExternalInput')
    dwpack = nc.dram_tensor('dwpack', [128, 9 * 2 * 2 * 128], BF16, kind='ExternalInput')
    basei = nc.dram_tensor('basei', [128, 512], F32, kind='ExternalInput')
    mcomb = nc.dram_tensor('mcomb', [128, 4 * 9], F32, kind='ExternalInput')
    bias_om = nc.dram_tensor('bias_om', [128, 1], F32, kind='ExternalInput')
    bias_out = nc.dram_tensor('bias_out', [256, 1], F32, kind='ExternalInput')
    out = nc.dram_tensor('out', [256, 2048], F32, kind='ExternalOutput')

    with tile.TileContext(nc) as tc, ExitStack() as ctx:
        sb = ctx.enter_context(tc.tile_pool(name='sb', bufs=1))
        sbm = ctx.enter_context(tc.tile_pool(name='sbm', bufs=1))
        sbt = ctx.enter_context(tc.tile_pool(name='sbt', bufs=2))
        sba = ctx.enter_context(tc.tile_pool(name='sba', bufs=4))
        gpool = ctx.enter_context(tc.tile_pool(name='gpool', bufs=8))
        spool = ctx.enter_context(tc.tile_pool(name='spool', bufs=2))

        ident = sb.tile([128, 128], BF16, tag='ident', name='ident')
        make_identity(nc, ident[:])
        identf = sb.tile([128, 128], F32, tag='identf', name='identf')
        make_identity(nc, identf[:])

        wp = sb.tile([128, 4 * 9 * 32], BF16, tag='wp', name='wp')
        nc.sync.dma_start(wp[:], wpack[:])
        dw = sb.tile([128, 9 * 2 * 2 * 128], BF16, tag='dw', name='dw')
        nc.sync.dma_start(dw[:], dwpack[:])
        base_sb = sb.tile([128, 512], F32, tag='base', name='base')
        nc.sync.dma_start(base_sb[:], basei[:])
        mc_sb = sb.tile([128, 4 * 9], F32, tag='mcomb', name='mcomb')
        nc.sync.dma_start(mc_sb[:], mcomb[:])
        bom_sb = sb.tile([128, 1], F32, tag='bom', name='bom')
        nc.sync.dma_start(bom_sb[:], bias_om[:])
        bout_sb = sb.tile([128, 2], F32, tag='bout', name='bout')
        nc.sync.dma_start(bout_sb[:], bias_out[:].rearrange('(g p) o -> p (g o)', g=2))

        # ---- stage 1: upsample template + build combined [512ch, 34, 66] ----
        with tc.tile_pool(name='convsb', bufs=1) as convsb, \
             tc.tile_pool(name='psA', bufs=1, space='PSUM') as psA:
            comb = []
            for cg in range(2):
                tp = convsb.tile([128, 23 * 32], BF16, tag=f'tp{cg}', name=f'tp{cg}')
                nc.sync.dma_start(tp[:], tplp[128 * cg:128 * (cg + 1), :])
                tp3 = tp[:].rearrange('p (r w) -> p r w', r=23)
                V = convsb.tile([128, 34 * 32], BF16, tag=f'vt{cg}', name=f'vt{cg}')
                V3 = V[:].rearrange('p (r w) -> p r w', r=34)
                tmp = convsb.tile([128, 34 * 32], BF16, tag=f'ut{cg}', name=f'ut{cg}')
                tmp3 = tmp[:].rearrange('p (r w) -> p r w', r=34)
                nc.vector.tensor_scalar_mul(tmp3[:, 0:16, :], tp3[:, 2:18, :], 0.25)
                nc.scalar.activation(V3[:, 2:34:2, :], tp3[:, 1:17, :], ACTF.Identity, scale=0.75)
                nc.vector.tensor_tensor(V3[:, 2:34:2, :], V3[:, 2:34:2, :], tmp3[:, 0:16, :], ALU.add)
                nc.vector.tensor_scalar_mul(tmp3[:, 0:16, :], tp3[:, 1:17, :], 0.75)
                nc.scalar.activation(V3[:, 1:33:2, :], tp3[:, 0:16, :], ACTF.Identity, scale=0.25)
                nc.vector.tensor_tensor(V3[:, 1:33:2, :], V3[:, 1:33:2, :], tmp3[:, 0:16, :], ALU.add)
                nc.vector.tensor_scalar_mul(tmp3[:, 0:1, :], tp3[:, 20:21, :], 0.25)
                nc.scalar.activation(V3[:, 0:1, :], tp3[:, 19:20, :], ACTF.Identity, scale=0.75)
                nc.vector.tensor_tensor(V3[:, 0:1, :], V3[:, 0:1, :], tmp3[:, 0:1, :], ALU.add)
                nc.vector.tensor_scalar_mul(tmp3[:, 0:1, :], tp3[:, 22:23, :], 0.75)
                nc.scalar.activation(V3[:, 33:34, :], tp3[:, 21:22, :], ACTF.Identity, scale=0.25)
                nc.vector.tensor_tensor(V3[:, 33:34, :], V3[:, 33:34, :], tmp3[:, 0:1, :], ALU.add)
                cb = convsb.tile([128, 34 * 66], BF16, tag=f'comb{cg}', name=f'comb{cg}')
                cb3 = cb[:].rearrange('p (r w) -> p r w', r=34)
                nc.vector.memset(cb[:], 0.0)
                h3 = tmp3
                nc.vector.tensor_scalar_mul(h3[:, :, 0:31], V3[:, :, 1:32], 0.75)
                nc.scalar.activation(cb3[:, :, 3:65:2], V3[:, :, 0:31], ACTF.Identity, scale=0.25)
                nc.vector.tensor_tensor(cb3[:, :, 3:65:2], cb3[:, :, 3:65:2], h3[:, :, 0:31], ALU.add)
                nc.vector.tensor_scalar_mul(h3[:, :, 0:31], V3[:, :, 1:32], 0.25)
                nc.scalar.activation(cb3[:, :, 2:64:2], V3[:, :, 0:31], ACTF.Identity, scale=0.75)
                nc.vector.tensor_tensor(cb3[:, :, 2:64:2], cb3[:, :, 2:64:2], h3[:, :, 0:31], ALU.add)
                nc.vector.tensor_copy(cb3[:, :, 1:2], V3[:, :, 0:1])
                nc.vector.tensor_copy(cb3[:, :, 64:65], V3[:, :, 31:32])
                comb.append(cb)
            for cg in range(2):
                cb = convsb.tile([128, 34 * 66], BF16, tag=f'comb{cg+2}', name=f'comb{cg+2}')
                nc.sync.dma_start(cb[:], srch66[128 * cg:128 * (cg + 1), :])
                comb.append(cb)

            # ---- stage 2: offsets+mask conv, col-tiled quarters ----
            wp4 = wp[:].rearrange('p (g t m) -> p g t m', g=4, t=9)
            pom = psA.tile([128, 512], F32, name='pom')
            for q in range(4):
                first = True
                for gi, g in enumerate((2, 3, 0, 1)):
                    cb3 = comb[g][:].rearrange('p (r w) -> p r w', r=34)
                    for t, (ky, kx) in enumerate(TAPS):
                        rhs = cb3[:, 8 * q + 1 + ky: 8 * q + 9 + ky, 1 + kx: 65 + kx]
                        nc.tensor.matmul(
                            pom[32 * q:32 * q + 32, :], wp4[:, g, t, :], rhs,
                            start=first, stop=(gi == 3 and t == 8),
                            tile_position=(0, 32 * q))
                        first = False
            om = sb.tile([128, 512], F32, tag='om', name='om')
            nc.scalar.activation(om[:], pom[:], ACTF.Identity, bias=bom_sb[:], scale=1.0)

        sg = sb.tile([128, 512], F32, tag='sg', name='sg')
        nc.scalar.activation(sg[:], om[:], ACTF.Sigmoid)

        # ---- stage 3: index math (fp32) ----
        def mtile(tag, dt=F32):
            return sbm.tile([128, 512], dt, tag=tag, name=tag)
        P = mtile('P')
        nc.vector.tensor_tensor(P[:], om[:], base_sb[:], ALU.add)
        nc.vector.tensor_scalar(P[:], P[:], CLAMP_HI, CLAMP_LO, ALU.min, ALU.max)
        T32 = mtile('T32', I32)
        nc.vector.tensor_copy(T32[:], P[:])
        Tf = mtile('Tf')
        nc.vector.tensor_copy(Tf[:], T32[:])
        Gg = mtile('Gg')
        nc.vector.tensor_tensor(Gg[:], Tf[:], P[:], ALU.is_gt)
        nc.vector.tensor_tensor(Tf[:], Tf[:], Gg[:], ALU.subtract)   # floor
        nc.vector.tensor_tensor(P[:], P[:], Tf[:], ALU.subtract)     # frac
        Wf = P

        # ---- stage 4a: idxf [9, 2048] = 66*(y0+1) + (x0+1) ----
        idxf = sb.tile([9, 2048], F32, tag='idxf', name='idxf')
        with tc.tile_pool(name='psI', bufs=2, space='PSUM') as psI:
            for q in range(4):
                pidx = psI.tile([9, 512], F32, name='pidx')
                nc.tensor.matmul(pidx[:], mc_sb[:, 9 * q:9 * (q + 1)], Tf[:],
                                 start=True, stop=True)
                nc.vector.tensor_scalar(idxf[:, 512 * q:512 * (q + 1)], pidx[:],
                                        67.0, None, ALU.add)
        # wrap: idx16[j, 128*t + 8*bb + a] = idxf[t, 128*bb + 16*a + j]
        idx16 = sb.tile([128, 9 * 128], I16, tag='idx16', name='idx16')
        tsb = sb.tile([128, 16 * 9], F32, tag='tsb', name='tsb')
        tsb3 = tsb[:].rearrange('p (b t) -> p b t', b=16)
        with tc.tile_pool(name='psT', bufs=2, space='PSUM') as psT:
            for bb in range(16):
                pT = psT.tile([128, 9], F32, name='pT')
                nc.tensor.transpose(pT[:], idxf[:, 128 * bb:128 * (bb + 1)],
                                    identf[0:9, 0:9], tile_position=(0, 0))
                nc.vector.tensor_copy(tsb3[:, bb, :], pT[:])
            for a in range(8):
                pW = psT.tile([16, 144], F32, name='pW')
                nc.tensor.matmul(pW[:], identf[:, 16 * a:16 * (a + 1)], tsb[:],
                                 start=True, stop=True)
                dsta = AP(idx16[:].tensor, idx16[:].offset + a,
                          [[9 * 128, 16], [8, 16], [128, 9]])
                nc.vector.tensor_copy(dsta, pW[:].rearrange('p (b t) -> p b t', b=16))
        nc.sync.dma_start(idx16[16:32, :], idx16[0:16, :])
        nc.sync.dma_start(idx16[32:64, :], idx16[0:32, :])
        nc.sync.dma_start(idx16[64:128, :], idx16[0:64, :])

        # ---- stage 4b: corner weights wq[128px, bb, j, t] ----
        # transpose (wy, wx, sg-mask) per 128-px block into taW, then compute
        # the 4 bilinear*mask products as strided full-tile ops over all 16
        # blocks at once.
        wq = sb.tile([128, 16 * 4 * 9], F32, tag='wq', name='wq')
        wq4 = wq[:].rearrange('p (b j t) -> p b j t', b=16, j=4)
        taW = sb.tile([128, 16 * 45], F32, tag='taW', name='taW')
        ta4 = taW[:].rearrange('p (b c) -> p b c', b=16)
        with tc.tile_pool(name='psW', bufs=2, space='PSUM') as psW:
            for bb in range(16):
                q, cc = bb // 4, bb % 4
                pt = psW.tile([128, 45], F32, name='ptw')
                idq18 = identf[32 * q:32 * q + 18, 32 * q:32 * q + 18]
                idq27 = identf[32 * q:32 * q + 27, 32 * q:32 * q + 27]
                nc.tensor.transpose(pt[:, 0:18],
                                    Wf[32 * q:32 * q + 18, 128 * cc:128 * (cc + 1)],
                                    idq18, tile_position=(32 * q, 0))
                nc.tensor.transpose(pt[:, 18:45],
                                    sg[32 * q:32 * q + 27, 128 * cc:128 * (cc + 1)],
                                    idq27, tile_position=(32 * q, 0))
                nc.vector.tensor_copy(ta4[:, bb, :], pt[:])
        # ta cols per block: 0:9 = wy, 9:18 = wx, 36:45 = sigmoid(mask)
        wy_ap = ta4[:, :, 0:9]
        wx_ap = ta4[:, :, 9:18]
        mk_ap = ta4[:, :, 36:45]
        iw = sb.tile([128, 16 * 2 * 9], F32, tag='iw', name='iw')
        iw4 = iw[:].rearrange('p (b j t) -> p b j t', b=16, j=2)
        iwy_ap = iw4[:, :, 0, :]
        iwx_ap = iw4[:, :, 1, :]
        nc.vector.tensor_scalar(iwy_ap, wy_ap, -1.0, 1.0, ALU.mult, ALU.add)
        nc.vector.tensor_scalar(iwx_ap, wx_ap, -1.0, 1.0, ALU.mult, ALU.add)
        q1t = sb.tile([128, 16 * 9], F32, tag='q1t', name='q1t')
        q13 = q1t[:].rearrange('p (b t) -> p b t', b=16)
        q2t = sb.tile([128, 16 * 9], F32, tag='q2t', name='q2t')
        q23 = q2t[:].rearrange('p (b t) -> p b t', b=16)
        nc.vector.tensor_tensor(q13[:, :, :], iwy_ap, mk_ap, ALU.mult)
        nc.vector.tensor_tensor(q23[:, :, :], wy_ap, mk_ap, ALU.mult)
        nc.vector.tensor_tensor(wq4[:, :, 0, :], q13[:, :, :], iwx_ap, ALU.mult)
        nc.vector.tensor_tensor(wq4[:, :, 1, :], q13[:, :, :], wx_ap, ALU.mult)
        nc.vector.tensor_tensor(wq4[:, :, 2, :], q23[:, :, :], iwx_ap, ALU.mult)
        nc.vector.tensor_tensor(wq4[:, :, 3, :], q23[:, :, :], wx_ap, ALU.mult)

        # ---- stages 5-8: per image half: gather -> blend -> transpose ->
        #      einsum (og0 inline per tap, og1 per half as tail) ----
        ST = sb.tile([128, 18 * 1024], BF16, tag='ST', name='ST')
        ST3 = ST[:].rearrange('p (k n) -> p k n', k=18)
        inap = AP(qtab[:].tensor, 0, [[1024, 4356], [1, 1024]])
        dw4 = dw[:].rearrange('p (k g o c) -> p k g o c', k=9, g=2, o=2)
        with tc.tile_pool(name='psQ', bufs=2, space='PSUM') as psQ, \
             tc.tile_pool(name='psO', bufs=1, space='PSUM') as psO, \
             tc.tile_pool(name='psO1', bufs=2, space='PSUM') as psO1:
            for hb in range(2):
                po0 = [psO.tile([128, 512], F32, tag=f'po0_{i}', name=f'po0_{i}')
                       for i in range(2)]
                for t in range(9):
                    gs = []
                    for sub in range(2):
                        g = gpool.tile([128, 4, 1024], BF16, tag='gt', name='gt')
                        nc.gpsimd.dma_gather(
                            out_ap=g[:], in_ap=inap,
                            idxs_ap=idx16[:, 128 * t + 64 * hb + 32 * sub:
                                          128 * t + 64 * hb + 32 * (sub + 1)],
                            num_idxs=512, num_idxs_reg=512,
                            elem_size=1024, elem_step=1024,
                            single_packet=False,
                            queue_num=(2 * (9 * hb + t) + sub) % 4)
                        gs.append(g)
                    S = spool.tile([128, 8, 256], BF16, tag='S', name='S')
                    S3 = S[:]
                    for blk in range(8):
                        bb = 8 * hb + blk
                        g3 = gs[blk // 4][:]
                        sblk = blk % 4
                        m = sba.tile([128, 256], BF16, tag='m', name='m')
                        nc.scalar.activation(m[:], g3[:, sblk, 0:256], ACTF.Identity,
                                             scale=wq4[:, bb, 0, t:t + 1])
                        nc.vector.scalar_tensor_tensor(
                            m[:], g3[:, sblk, 256:512], wq4[:, bb, 1, t:t + 1], m[:],
                            ALU.mult, ALU.add)
                        nc.vector.scalar_tensor_tensor(
                            m[:], g3[:, sblk, 512:768], wq4[:, bb, 2, t:t + 1], m[:],
                            ALU.mult, ALU.add)
                        nc.vector.scalar_tensor_tensor(
                            S3[:, blk, :], g3[:, sblk, 768:1024], wq4[:, bb, 3, t:t + 1],
                            m[:], ALU.mult, ALU.add)
                    for cg in range(2):
                        pq = psQ.tile([128, 1024], BF16, name='pq')
                        for blk in range(8):
                            nc.tensor.transpose(
                                pq[:, 128 * blk:128 * (blk + 1)],
                                S3[:, blk, 128 * cg:128 * (cg + 1)], ident[:])
                        nc.scalar.activation(ST3[:, 2 * t + cg, :], pq[:],
                                             ACTF.Identity)
                        for i in range(2):
                            nc.tensor.matmul(
                                po0[i][:], dw4[:, t, cg, 0, :],
                                ST3[:, 2 * t + cg, 512 * i:512 * (i + 1)],
                                start=(t == 0 and cg == 0), stop=(t == 8 and cg == 1))
                for i in range(2):
                    q = 2 * hb + i
                    osb = sbt.tile([128, 512], F32, tag='osb', name='osb')
                    nc.scalar.activation(osb[:], po0[i][:], ACTF.Identity,
                                         bias=bout_sb[:, 0:1], scale=1.0)
                    nc.sync.dma_start(out[0:128, 512 * q:512 * (q + 1)], osb[:])
                    po = psO1.tile([128, 512], F32, name='po')
                    for t in range(9):
                        for cg in range(2):
                            nc.tensor.matmul(
                                po[:], dw4[:, t, cg, 1, :],
                                ST3[:, 2 * t + cg, 512 * i:512 * (i + 1)],
                                start=(t == 0 and cg == 0), stop=(t == 8 and cg == 1))
                    osb1 = sbt.tile([128, 512], F32, tag='osb1', name='osb1')
                    nc.scalar.activation(osb1[:], po[:], ACTF.Identity,
                                         bias=bout_sb[:, 1:2], scale=1.0)
                    nc.sync.dma_start(out[128:256, 512 * q:512 * (q + 1)], osb1[:])

    nc.compile()
    return nc


def _bf16(x):
    import ml_dtypes
    return np.asarray(x, dtype=np.float32).astype(ml_dtypes.bfloat16)


def prep_sample(inputs, b):
    """Per-sample (shared by both h-halves) heavy prep: the quad table."""
    sf = np.ascontiguousarray(np.asarray(inputs['search_feat'][b], dtype=np.float32))
    P = np.zeros((67, 67, 256), np.float32)
    P[1:65, 1:65] = sf.transpose(1, 2, 0)
    Q = np.concatenate([P[:66, :66], P[:66, 1:67], P[1:67, :66], P[1:67, 1:67]],
                       axis=-1)
    return _bf16(Q.reshape(4356, 1024))


def prep_core_inputs(inputs, b, h, qtab):
    tf = np.ascontiguousarray(np.asarray(inputs['template_feat'][b], dtype=np.float32))
    sf = np.ascontiguousarray(np.asarray(inputs['search_feat'][b], dtype=np.float32))
    offset_w = np.asarray(inputs['offset_w'], dtype=np.float32)
    offset_b = np.asarray(inputs['offset_b'], dtype=np.float32)
    mask_w = np.asarray(inputs['mask_w'], dtype=np.float32)
    mask_b = np.asarray(inputs['mask_b'], dtype=np.float32)
    deform_w = np.asarray(inputs['deform_w'], dtype=np.float32)
    deform_b = np.asarray(inputs['deform_b'], dtype=np.float32)

    tplp = np.zeros((256, 23, 32), np.float32)
    for j in range(19):
        tplp[:, j] = tf[:, min(max(16 * h - 1 + j, 0), 31)]
    if h == 0:
        tplp[:, 21] = tf[:, 15]
        tplp[:, 22] = tf[:, 16]
    else:
        tplp[:, 19] = tf[:, 15]
        tplp[:, 20] = tf[:, 16]

    srch66 = np.zeros((256, 34, 66), np.float32)
    for i in range(34):
        r = 32 * h - 1 + i
        if 0 <= r <= 63:
            srch66[:, i, 1:65] = sf[:, r]

    wpack = np.zeros((128, 4, 9, 32), np.float32)
    for g in range(4):
        for t, (ky, kx) in enumerate(TAPS):
            cs = slice(128 * g, 128 * (g + 1))
            wpack[:, g, t, 0:9] = offset_w[0::2, cs, ky + 1, kx + 1].T
            wpack[:, g, t, 9:18] = offset_w[1::2, cs, ky + 1, kx + 1].T
            if ky == 0 and kx == 0:
                wpack[:, g, t, 18:27] = mask_w[:, cs, 0, 0].T
    wk = deform_w.reshape(256, 256, 3, 3)
    dwp = np.zeros((128, 9, 2, 2, 128), np.float32)
    for t in range(9):
        ky, kx = TAPS[t]
        for cg in range(2):
            for og in range(2):
                dwp[:, t, cg, og, :] = wk[128 * og:128 * (og + 1),
                                          128 * cg:128 * (cg + 1), ky + 1, kx + 1].T

    basei = np.zeros((128, 512), np.float32)
    col = np.arange(512)
    for q in range(4):
        for m in range(9):
            basei[32 * q + m] = 32 * h + 8 * q + col // 64 + TAPS[m][0]
            basei[32 * q + 9 + m] = col % 64 + TAPS[m][1]

    mcomb = np.zeros((128, 4, 9), np.float32)
    for q in range(4):
        for t in range(9):
            mcomb[32 * q + t, q, t] = 66.0
            mcomb[32 * q + 9 + t, q, t] = 1.0

    bias_om = np.zeros((128, 1), np.float32)
    for q in range(4):
        bias_om[32 * q + 0:32 * q + 9, 0] = offset_b[0::2]
        bias_om[32 * q + 9:32 * q + 18, 0] = offset_b[1::2]
        bias_om[32 * q + 18:32 * q + 27, 0] = mask_b

    return {
        'tplp': _bf16(tplp.reshape(256, 23 * 32)),
        'srch66': _bf16(srch66.reshape(256, 34 * 66)),
        'qtab': qtab,
        'wpack': _bf16(wpack.reshape(128, 4 * 9 * 32)),
        'dwpack': _bf16(dwp.reshape(128, 9 * 2 * 2 * 128)),
        'basei': basei,
        'mcomb': mcomb.reshape(128, 4 * 9),
        'bias_om': bias_om,
        'bias_out': deform_b.reshape(256, 1).astype(np.float32),
    }


def kernel(**inputs):
    key = 'v2'
    if key not in _NC_CACHE:
        _NC_CACHE[key] = build_nc()
    nc = _NC_CACHE[key]
    qtabs = [prep_sample(inputs, b) for b in range(4)]
    in_maps = [prep_core_inputs(inputs, ci // 2, ci % 2, qtabs[ci // 2])
               for ci in range(8)]
    res = run_bass_kernel_spmd(nc, in_maps, core_ids=list(range(8)))
    global LAST_RESULT
    LAST_RESULT = res
    out = np.zeros((4, 256, 64, 64), np.float32)
    for ci in range(8):
        b, h = ci // 2, ci % 2
        out[b][:, 32 * h:32 * h + 32, :] = res.results[ci]['out'].reshape(256, 32, 64)
    return out
